# revision 8
# baseline (speedup 1.0000x reference)
# Mistral-style GQA attention layer (QKV proj + RoPE + causal attention +
# o_proj), tensor-parallel over heads across 8 NeuronCores.
#
# Sharding (8-way TP over heads): core c owns q heads [4c..4c+4) and kv head c.
#   - w_qkv rows sharded: 4 q-head blocks + 1 k block + 1 v block per core
#   - w_o columns sharded: each core computes a partial o_proj output.
#
# Host<->device traffic is minimized: every input byte is uploaded exactly
# once (fp16), and every output byte downloaded exactly once:
#   - x is uploaded token-sharded (1/8 per core) and AllGathered on device
#     (split in two along the contraction axis so phase 1 can start on the
#     first half while the second gathers).
#   - o_proj partials are ReduceScattered on device per batch (4 chunks,
#     pipelined behind compute), so each core returns a 512-row scramble of
#     the final output that the host reassembles by slicing.
#   - identity/ones/causal-mask tables are generated on device.
#
# Device kernel (identical SPMD program, per-core data):
#   phase 0: 2x AllGather of the token-shards of x^T.
#   phase 1: qkvT = Wc @ X^T (outputs TRANSPOSED: [dim, t]) + inline RoPE on
#            q/k rows, spilled to DRAM scratch.
#   phase 2 (per batch): per head: S^T = K Q^T on PE, exp on ACT (no max-sub:
#            scores are O(5) and fp32 exp is safe), causal mask via
#            multiplicative 0/1 tiles on DVE, softmax denominator via
#            ones-matmul over the partition (key) axis, P@V with V as the
#            stationary operand (needs V natural layout -> 128x128 PE
#            transposes of V^T), normalize at the end. Then o_partial rows
#            for this batch (attn @ Wo_c^T) and the batch's ReduceScatter.
#
# All matmuls run in fp16 (full PE rate, fp32 PSUM accumulate); inputs are
# quantized to fp16 on host (~1e-3 relative error, well inside tolerance).

import os
from contextlib import nullcontext
from dataclasses import dataclass

import numpy as np

import concourse.bass as bass
from concourse import bacc
import concourse.mybir as mybir
import concourse.tile as tile
from concourse.bass import ds, ts
from concourse.bass_utils import run_bass_kernel_spmd
from concourse.masks import make_identity

F32 = mybir.dt.float32
F32R = mybir.dt.float32r
EXPF = mybir.ActivationFunctionType.Exp
F16 = mybir.dt.float16


@dataclass(frozen=True)
class Cfg:
    T: int = 4096          # total tokens (B*S)
    H: int = 4096          # hidden size
    S: int = 1024          # seq len
    nq: int = 4            # q heads per core
    n_cores: int = 8
    D: int = 128           # head dim
    mm16: bool = True      # fp16 matmul paths instead of fp32r

    @property
    def B(self):
        return self.T // self.S

    @property
    def TOK(self):  # token shard per core
        return self.T // self.n_cores

    @property
    def KO(self):  # contraction tiles for qkv proj
        return self.H // 128

    @property
    def NM(self):  # qkv output row-tiles per core (q heads + k + v)
        return self.nq + 2

    @property
    def QW(self):  # q tile width in attention
        return min(512, self.S)

    @property
    def NJ(self):
        return self.S // self.QW

    @property
    def NKT(self):  # key tiles per batch
        return self.S // 128


FULL = Cfg()


def build_nc(cfg: Cfg, loop: int | None = None, phases=(0, 1, 2, 3)) -> bass.Bass:
    nc = bacc.Bacc("TRN2", target_bir_lowering=False, debug=False, num_devices=cfg.n_cores)
    MDT = F16 if cfg.mm16 else F32R          # matmul-path storage dtype
    TDT = F16 if cfg.mm16 else F32           # table dtype (cos/sin/masks)
    T, H, S, nq, D = cfg.T, cfg.H, cfg.S, cfg.nq, cfg.D
    KO, NM, QW, NJ, NKT, B = cfg.KO, cfg.NM, cfg.QW, cfg.NJ, cfg.NKT, cfg.B
    TOK, NC = cfg.TOK, cfg.n_cores
    KO2 = KO // 2  # AllGather half (k-tiles)
    NRT = QW // 128  # number of diagonal mask offsets
    scale = 1.0 / np.sqrt(D)
    groups = [list(range(NC))]

    xT = nc.dram_tensor("xT", [KO, 128, TOK], MDT, kind="ExternalInput")
    wqkvT = nc.dram_tensor("wqkvT", [KO, 128, NM * 128], MDT, kind="ExternalInput")
    woT = nc.dram_tensor("woT", [nq, 128, H], MDT, kind="ExternalInput")
    cosT = nc.dram_tensor("cosT", [128, S], TDT, kind="ExternalInput")
    sinT = nc.dram_tensor("sinT", [128, S], TDT, kind="ExternalInput")
    out = nc.dram_tensor("o_sh", [TOK, H], F16, kind="ExternalOutput")

    with tile.TileContext(nc) as tc:
        with (
            tc.tile_pool(name="psum", bufs=8, space="PSUM") as psum,
            tc.tile_pool(name="consts", bufs=1) as consts,
            tc.tile_pool(name="dram", bufs=1, space="DRAM") as dram,
        ):
            qkv_sp = dram.tile([NM, 128, T], MDT)
            xTg1 = dram.tile([NC, KO2, 128, TOK], MDT, addr_space="Shared")
            xTg2 = dram.tile([NC, KO2, 128, TOK], MDT, addr_space="Shared")
            o_sp = dram.tile([T, H], F16)
            b_rso = dram.tile([TOK, H], F16)

            with (tc.For_i(0, loop, 1) if loop else nullcontext()):
                if 0 in phases:
                    # -------- phase 0: AllGather token-shards of x^T --------
                    b_xin = dram.tile([KO, 128, TOK], MDT, tag="b_xin")
                    nc.sync.dma_start(b_xin[ds(0, KO2)], xT[ds(0, KO2)])
                    nc.sync.dma_start(b_xin[ds(KO2, KO2)], xT[ds(KO2, KO2)])
                    nc.gpsimd.collective_compute(
                        "AllGather",
                        mybir.AluOpType.bypass,
                        replica_groups=groups,
                        ins=[b_xin[ds(0, KO2), :, :].opt()],
                        outs=[xTg1[:].opt()],
                    )
                    nc.gpsimd.collective_compute(
                        "AllGather",
                        mybir.AluOpType.bypass,
                        replica_groups=groups,
                        ins=[b_xin[ds(KO2, KO2), :, :].opt()],
                        outs=[xTg2[:].opt()],
                    )

                # device-generated tables (after the CC triggers so the
                # gathers start as early as possible)
                ident = consts.tile([128, 128], MDT)
                make_identity(nc, ident)
                ones = consts.tile([128, 1], MDT)
                nc.gpsimd.memset(ones, 1.0)
                ones_row = consts.tile([1, 128], F32)
                nc.gpsimd.memset(ones_row, 1.0)
                masks_sb = consts.tile([128, NRT, QW], TDT)
                nc.gpsimd.memset(masks_sb, 1.0)
                for r in range(NRT):
                    # masks_sb[p, r, q] = (q - p - 128 r) >= 0 ? 1 : 0
                    nc.gpsimd.affine_select(
                        out=masks_sb[:, r, :],
                        in_=masks_sb[:, r, :],
                        compare_op=mybir.AluOpType.is_ge,
                        fill=0.0,
                        base=-128 * r,
                        pattern=[[1, QW]],
                        channel_multiplier=-1,
                    )
                nbias = consts.tile([128, 1], F32)
                nc.gpsimd.memset(nbias, -4.0)

                if 1 in phases:
                    # ---------------- phase 1: QKV projection + RoPE ----------------
                    with (
                        tc.tile_pool(name="wq", bufs=1) as wq_pool,
                        tc.tile_pool(name="xin", bufs=2) as xin,
                        tc.tile_pool(name="stage", bufs=2) as stage,
                        tc.tile_pool(name="rot", bufs=2) as rot_pool,
                        tc.tile_pool(name="tab", bufs=1) as tab,
                    ):
                        w_all = wq_pool.tile([128, KO, NM * 128], MDT)
                        nc.sync.dma_start(w_all, wqkvT[:].rearrange("k p m -> p k m"))
                        cos_sb = tab.tile([128, S], TDT)
                        nc.sync.dma_start(cos_sb, cosT[:])
                        sin_sb = tab.tile([128, S], TDT)
                        nc.sync.dma_start(sin_sb, sinT[:])

                        SLAB = TOK  # one gathered token-shard per slab
                        KH = min(8, KO)  # k-tiles per x-slab chunk
                        NCH = KO // KH
                        RH = min(256, SLAB)  # RoPE column-chunk
                        for n in range(cfg.T // SLAB):
                            tsl = ds(n * SLAB, SLAB)
                            # PSUM tiles first so matmuls can start per-chunk
                            pss = [
                                psum.tile([128, SLAB], F32, tag="bank", name=f"qk_ps{m}")
                                for m in range(NM)
                            ]
                            for ch in range(NCH):
                                xt = xin.tile([128, KH, SLAB], MDT, tag="xh")
                                src = (
                                    xTg1[n, ds(ch * KH, KH), :, :]
                                    if (ch * KH) < KO2
                                    else xTg2[n, ds(ch * KH - KO2, KH), :, :]
                                )
                                nc.sync.dma_start(xt, src.rearrange("k p t -> p k t"))
                                for m in range(NM):
                                    for k in range(KH):
                                        nc.tensor.matmul(
                                            pss[m],
                                            w_all[:, ch * KH + k, ts(m, 128)],
                                            xt[:, k, :],
                                            start=(ch == 0 and k == 0),
                                            stop=(ch == NCH - 1 and k == KH - 1),
                                        )
                            st = stage.tile([128, NM, SLAB], MDT)
                            for m in range(NM):
                                nc.scalar.copy(st[:, m, :], pss[m])
                            # RoPE on q heads + k head (rows 0..nq), not v
                            for rh in range(SLAB // RH):
                                rsl = ds(rh * RH, RH)
                                rot = rot_pool.tile([128, nq + 1, RH], MDT, tag="rot")
                                nc.sync.dma_start(rot[0:64], st[64:128, 0 : nq + 1, rsl])
                                nc.sync.dma_start(rot[64:128], st[0:64, 0 : nq + 1, rsl])
                                s0 = (n * SLAB + rh * RH) % S
                                cos_b = cos_sb[:, None, ds(s0, RH)].to_broadcast(
                                    (128, nq + 1, RH)
                                )
                                sin_b = sin_sb[:, None, ds(s0, RH)].to_broadcast(
                                    (128, nq + 1, RH)
                                )
                                nc.vector.tensor_mul(
                                    st[:, 0 : nq + 1, rsl], st[:, 0 : nq + 1, rsl], cos_b
                                )
                                nc.vector.tensor_mul(rot, rot, sin_b)
                                nc.vector.tensor_add(
                                    st[:, 0 : nq + 1, rsl], st[:, 0 : nq + 1, rsl], rot
                                )
                            nc.sync.dma_start(
                                qkv_sp[:, :, tsl].rearrange("m p t -> p m t"), st
                            )

                if 2 in phases:
                    # ------- phase 2: attention + per-batch o_proj + RS -------
                    with (
                        tc.tile_pool(name="attn", bufs=2) as attn_pool,
                        tc.tile_pool(name="kv", bufs=2) as kv_pool,
                        tc.tile_pool(name="vn", bufs=2) as vn_pool,
                        tc.tile_pool(name="qh", bufs=3) as q_pool,
                        tc.tile_pool(name="pt", bufs=12) as pt_pool,
                        tc.tile_pool(name="sm", bufs=4) as sm_pool,
                        tc.tile_pool(name="wo", bufs=1) as wo_pool,
                        tc.tile_pool(name="ot", bufs=4) as ot_pool,
                    ):
                        wo_all = wo_pool.tile([128, nq, H], MDT)
                        nc.sync.dma_start(wo_all, woT[:].rearrange("a p n -> p a n"))
                        for b in range(B):
                            bsl = ds(b * S, S)
                            attnT = attn_pool.tile([128, nq, S], MDT, tag="attnT")
                            kT = kv_pool.tile([128, S], MDT, tag="kT")
                            nc.sync.dma_start(kT, qkv_sp[nq, :, bsl])
                            vT = kv_pool.tile([128, S], MDT, tag="vT")
                            nc.sync.dma_start(vT, qkv_sp[nq + 1, :, bsl])
                            vn = vn_pool.tile([128, NKT, 128], MDT)
                            for kt in range(NKT):
                                tp = psum.tile([128, 128], MDT, tag="bank")
                                nc.tensor.transpose(tp, vT[:, ts(kt, 128)], ident)
                                nc.vector.tensor_copy(vn[:, kt, :], tp)
                            qs = []
                            for h in range(nq):
                                q = q_pool.tile([128, S], MDT, name=f"q{h}")
                                nc.sync.dma_start(q, qkv_sp[h, :, bsl])
                                qs.append(q)
                            for j in range(NJ):
                                nkt = (j + 1) * NRT
                                qsl = ds(j * QW, QW)
                                for h in range(nq):
                                    pts = []
                                    for kt in range(nkt):
                                        sp = psum.tile([128, QW], F32, tag="bank")
                                        nc.tensor.matmul(
                                            sp,
                                            kT[:, ts(kt, 128)],
                                            qs[h][:, qsl],
                                            start=True,
                                            stop=True,
                                        )
                                        pt = pt_pool.tile([128, QW], MDT, tag="pt")
                                        nc.scalar.activation(pt, sp, EXPF, bias=nbias[:, 0:1], scale=scale)
                                        if kt >= j * NRT:
                                            nc.vector.tensor_mul(
                                                pt, pt, masks_sb[:, kt - j * NRT, :]
                                            )
                                        pts.append(pt)
                                    sum_ps = psum.tile([1, QW], F32, tag="bank")
                                    for i, pt in enumerate(pts):
                                        nc.tensor.matmul(
                                            sum_ps,
                                            ones,
                                            pt,
                                            start=(i == 0),
                                            stop=(i == len(pts) - 1),
                                        )
                                    recip = sm_pool.tile([1, QW], F32, tag="recip")
                                    nc.vector.reciprocal(recip, sum_ps)
                                    # broadcast 1/denom across partitions via a
                                    # rank-1 PE matmul (keeps gpsimd free for
                                    # the collective triggers)
                                    rb_ps = psum.tile([128, QW], F32, tag="bank")
                                    nc.tensor.matmul(
                                        rb_ps, ones_row, recip, start=True, stop=True
                                    )
                                    rb = sm_pool.tile([128, QW], F32, tag="rb")
                                    nc.scalar.copy(rb, rb_ps)
                                    pv = psum.tile([128, QW], F32, tag="bank")
                                    for i, pt in enumerate(pts):
                                        nc.tensor.matmul(
                                            pv,
                                            vn[:, i, :],
                                            pt,
                                            start=(i == 0),
                                            stop=(i == len(pts) - 1),
                                        )
                                    nc.vector.tensor_mul(
                                        attnT[:, h, ds(j * QW, QW)], pv, rb
                                    )

                                if 3 in phases:
                                    # ---- o_proj rows for this batch's j-block ----
                                    for tm in range(QW // 128):
                                        tloc = j * QW + tm * 128
                                        for nh in range(H // 512):
                                            hsl = ds(nh * 512, 512)
                                            ps = psum.tile([128, 512], F32, tag="bank")
                                            for a in range(nq):
                                                nc.tensor.matmul(
                                                    ps,
                                                    attnT[:, a, ds(tloc, 128)],
                                                    wo_all[:, a, hsl],
                                                    start=(a == 0),
                                                    stop=(a == nq - 1),
                                                )
                                            ot = ot_pool.tile([128, 512], F16)
                                            if (tm + nh) % 2 == 0:
                                                nc.scalar.copy(ot, ps)
                                            else:
                                                nc.vector.tensor_copy(ot, ps)
                                            nc.sync.dma_start(
                                                o_sp[ds(b * S + tloc, 128), hsl], ot
                                            )

                            if 3 in phases:
                                # ---- ReduceScatter this batch's partials ----
                                nc.gpsimd.collective_compute(
                                    "ReduceScatter",
                                    mybir.AluOpType.add,
                                    replica_groups=groups,
                                    ins=[o_sp[ds(b * S, S), :].opt()],
                                    outs=[b_rso[ds(b * 128, 128), :].opt()],
                                )
                                nc.sync.dma_start(
                                    out[ds(b * 128, 128), :],
                                    b_rso[ds(b * 128, 128), :],
                                )
    nc.compile()
    return nc


def prep_core_inputs(cfg: Cfg, hidden, w_qkv, w_o, core: int):
    """Build the per-core input map (C-contiguous, matmul-path dtype)."""
    ndt = np.float16 if cfg.mm16 else np.float32
    T, H, S, nq, D = cfg.T, cfg.H, cfg.S, cfg.nq, cfg.D
    TOK = cfg.TOK
    NQ_TOT = cfg.n_cores * nq
    xT = np.ascontiguousarray(
        hidden[core * TOK : (core + 1) * TOK, :].T.astype(ndt)
    ).reshape(cfg.KO, 128, TOK)
    q0 = core * nq * D
    rows = list(range(q0, q0 + nq * D))
    rows += list(range(NQ_TOT * D + core * D, NQ_TOT * D + (core + 1) * D))
    kv_heads = cfg.n_cores  # one kv head per core
    rows += list(
        range((NQ_TOT + kv_heads) * D + core * D, (NQ_TOT + kv_heads) * D + (core + 1) * D)
    )
    wqkvT = np.ascontiguousarray(w_qkv[rows, :].T.astype(ndt)).reshape(cfg.KO, 128, cfg.NM * 128)
    woT = np.ascontiguousarray(
        w_o[:, core * nq * D : (core + 1) * nq * D].T.astype(ndt)
    ).reshape(nq, 128, H)
    return {"xT": xT, "wqkvT": wqkvT, "woT": woT}


def prep_shared_inputs(cfg: Cfg, rope_theta=10000.0):
    ndt = np.float16 if cfg.mm16 else np.float32
    S, D = cfg.S, cfg.D
    inv = 1.0 / (rope_theta ** (np.arange(0, D, 2, dtype=np.float64) / D))
    ang = np.arange(S, dtype=np.float64)[:, None] * inv[None, :]  # [S, 64]
    cos = np.cos(ang).T.astype(np.float32)  # [64, S]
    sin = np.sin(ang).T.astype(np.float32)
    cosT = np.concatenate([cos, cos], axis=0)  # [128, S]
    sinT = np.concatenate([-sin, sin], axis=0)
    return {
        "cosT": np.ascontiguousarray(cosT.astype(ndt)),
        "sinT": np.ascontiguousarray(sinT.astype(ndt)),
    }


_CACHE = {}
LAST_EXEC_NS = None


def _get_nc(cfg: Cfg) -> bass.Bass:
    if cfg not in _CACHE:
        _CACHE[cfg] = build_nc(cfg)
    return _CACHE[cfg]


def kernel(hidden_states=None, w_qkv=None, w_o=None, seq_len=None, **_):
    cfg = FULL
    hidden = np.asarray(hidden_states, dtype=np.float32)
    w_qkv = np.asarray(w_qkv, dtype=np.float32)
    w_o = np.asarray(w_o, dtype=np.float32)

    nc = _get_nc(cfg)
    shared = prep_shared_inputs(cfg)
    in_maps = []
    for c in range(cfg.n_cores):
        m = dict(shared)
        m.update(prep_core_inputs(cfg, hidden, w_qkv, w_o, c))
        in_maps.append(m)

    trace = os.environ.get("KERNEL_TRACE", "0") == "1"
    res = run_bass_kernel_spmd(
        nc, in_maps, core_ids=list(range(cfg.n_cores)), trace=trace
    )
    global LAST_EXEC_NS
    if res.exec_time_ns is not None:
        LAST_EXEC_NS = res.exec_time_ns
        print(f"HW exec time: {res.exec_time_ns} ns")
        if res.instructions_and_trace is not None:
            print(f"trace: {res.instructions_and_trace[1]}")
    # Reassemble: RS chunk b on core c holds final rows [S*b + 128*c, +128).
    out_full = np.empty((cfg.T, cfg.H), dtype=np.float32)
    for c in range(cfg.n_cores):
        sh = res.results[c]["o_sh"]
        for b in range(cfg.B):
            out_full[cfg.S * b + 128 * c : cfg.S * b + 128 * c + 128] = sh[
                128 * b : 128 * b + 128
            ]
    return out_full


# revision 9
# speedup vs baseline: 1.1104x; 1.1104x over previous
# Mistral-style GQA attention layer (QKV proj + RoPE + causal attention +
# o_proj), tensor-parallel over heads across 8 NeuronCores.
#
# Sharding (8-way TP over heads): core c owns q heads [4c..4c+4) and kv head c.
#   - w_qkv rows sharded: 4 q-head blocks + 1 k block + 1 v block per core
#   - w_o columns sharded: each core computes a partial o_proj output.
#
# Host<->device traffic is minimized: every input byte is uploaded exactly
# once (fp16), and every output byte downloaded exactly once:
#   - x is uploaded token-sharded (1/8 per core) and AllGathered on device
#     (split in two along the contraction axis so phase 1 can start on the
#     first half while the second gathers).
#   - o_proj partials are ReduceScattered on device per batch (4 chunks,
#     pipelined behind compute), so each core returns a 512-row scramble of
#     the final output that the host reassembles by slicing.
#   - identity/ones/causal-mask tables are generated on device.
#
# Device kernel (identical SPMD program, per-core data):
#   phase 0: 2x AllGather of the token-shards of x^T.
#   phase 1: qkvT = Wc @ X^T (outputs TRANSPOSED: [dim, t]) + inline RoPE on
#            q/k rows, spilled to DRAM scratch.
#   phase 2 (per batch): per head: S^T = K Q^T on PE, exp on ACT (no max-sub:
#            scores are O(5) and fp32 exp is safe), causal mask via
#            multiplicative 0/1 tiles on DVE, softmax denominator via
#            ones-matmul over the partition (key) axis, P@V with V as the
#            stationary operand (needs V natural layout -> 128x128 PE
#            transposes of V^T), normalize at the end. Then o_partial rows
#            for this batch (attn @ Wo_c^T) and the batch's ReduceScatter.
#
# All matmuls run in fp16 (full PE rate, fp32 PSUM accumulate); inputs are
# quantized to fp16 on host (~1e-3 relative error, well inside tolerance).

import os
from contextlib import nullcontext
from dataclasses import dataclass

import numpy as np

import concourse.bass as bass
from concourse import bacc
import concourse.mybir as mybir
import concourse.tile as tile
from concourse.bass import ds, ts
from concourse.bass_utils import run_bass_kernel_spmd
from concourse.masks import make_identity

F32 = mybir.dt.float32
F32R = mybir.dt.float32r
EXPF = mybir.ActivationFunctionType.Exp
F16 = mybir.dt.float16


@dataclass(frozen=True)
class Cfg:
    T: int = 4096          # total tokens (B*S)
    H: int = 4096          # hidden size
    S: int = 1024          # seq len
    nq: int = 4            # q heads per core
    n_cores: int = 8
    D: int = 128           # head dim
    mm16: bool = True      # fp16 matmul paths instead of fp32r

    @property
    def B(self):
        return self.T // self.S

    @property
    def TOK(self):  # token shard per core
        return self.T // self.n_cores

    @property
    def KO(self):  # contraction tiles for qkv proj
        return self.H // 128

    @property
    def NM(self):  # qkv output row-tiles per core (q heads + k + v)
        return self.nq + 2

    @property
    def QW(self):  # q tile width in attention
        return min(512, self.S)

    @property
    def NJ(self):
        return self.S // self.QW

    @property
    def NKT(self):  # key tiles per batch
        return self.S // 128


FULL = Cfg()


def build_nc(cfg: Cfg, loop: int | None = None, phases=(0, 1, 2, 3)) -> bass.Bass:
    nc = bacc.Bacc("TRN2", target_bir_lowering=False, debug=False, num_devices=cfg.n_cores)
    MDT = F16 if cfg.mm16 else F32R          # matmul-path storage dtype
    TDT = F16 if cfg.mm16 else F32           # table dtype (cos/sin/masks)
    T, H, S, nq, D = cfg.T, cfg.H, cfg.S, cfg.nq, cfg.D
    KO, NM, QW, NJ, NKT, B = cfg.KO, cfg.NM, cfg.QW, cfg.NJ, cfg.NKT, cfg.B
    TOK, NC = cfg.TOK, cfg.n_cores
    KO2 = KO // 2  # AllGather half (k-tiles)
    NRT = QW // 128  # number of diagonal mask offsets
    scale = 1.0 / np.sqrt(D)
    groups = [list(range(NC))]

    xT = nc.dram_tensor("xT", [KO, 128, TOK], MDT, kind="ExternalInput")
    wqkvT = nc.dram_tensor("wqkvT", [KO, 128, NM * 128], MDT, kind="ExternalInput")
    woT = nc.dram_tensor("woT", [nq, 128, H], MDT, kind="ExternalInput")
    cosT = nc.dram_tensor("cosT", [128, S], TDT, kind="ExternalInput")
    sinT = nc.dram_tensor("sinT", [128, S], TDT, kind="ExternalInput")
    out = nc.dram_tensor("o_sh", [TOK, H], F16, kind="ExternalOutput")

    with tile.TileContext(nc) as tc:
        with (
            tc.tile_pool(name="psum", bufs=8, space="PSUM") as psum,
            tc.tile_pool(name="consts", bufs=1) as consts,
            tc.tile_pool(name="dram", bufs=1, space="DRAM") as dram,
        ):
            qkv_sp = dram.tile([NM, 128, T], MDT)
            xTg1 = dram.tile([NC, KO2, 128, TOK], MDT, addr_space="Shared")
            xTg2 = dram.tile([NC, KO2, 128, TOK], MDT, addr_space="Shared")
            o_sp = dram.tile([T, H], F16)
            b_rso = dram.tile([TOK, H], F16)

            with (tc.For_i(0, loop, 1) if loop else nullcontext()):
                if 0 in phases:
                    # -------- phase 0: AllGather token-shards of x^T --------
                    b_xin = dram.tile([KO, 128, TOK], MDT, tag="b_xin")
                    nc.sync.dma_start(b_xin[ds(0, KO2)], xT[ds(0, KO2)])
                    nc.sync.dma_start(b_xin[ds(KO2, KO2)], xT[ds(KO2, KO2)])
                    nc.gpsimd.collective_compute(
                        "AllGather",
                        mybir.AluOpType.bypass,
                        replica_groups=groups,
                        ins=[b_xin[ds(0, KO2), :, :].opt()],
                        outs=[xTg1[:].opt()],
                    )
                    nc.gpsimd.collective_compute(
                        "AllGather",
                        mybir.AluOpType.bypass,
                        replica_groups=groups,
                        ins=[b_xin[ds(KO2, KO2), :, :].opt()],
                        outs=[xTg2[:].opt()],
                    )

                # device-generated tables (after the CC triggers so the
                # gathers start as early as possible)
                ident = consts.tile([128, 128], MDT)
                make_identity(nc, ident)
                ones = consts.tile([128, 1], MDT)
                nc.gpsimd.memset(ones, 1.0)
                ones_row = consts.tile([1, 128], F32)
                nc.gpsimd.memset(ones_row, 1.0)
                masks_sb = consts.tile([128, NRT, QW], TDT)
                nc.gpsimd.memset(masks_sb, 1.0)
                for r in range(NRT):
                    # masks_sb[p, r, q] = (q - p - 128 r) >= 0 ? 1 : 0
                    nc.gpsimd.affine_select(
                        out=masks_sb[:, r, :],
                        in_=masks_sb[:, r, :],
                        compare_op=mybir.AluOpType.is_ge,
                        fill=0.0,
                        base=-128 * r,
                        pattern=[[1, QW]],
                        channel_multiplier=-1,
                    )
                nbias = consts.tile([128, 1], F32)
                nc.gpsimd.memset(nbias, -4.0)

                if 1 in phases:
                    # ---------------- phase 1: QKV projection + RoPE ----------------
                    with (
                        tc.tile_pool(name="wq", bufs=1) as wq_pool,
                        tc.tile_pool(name="xin", bufs=2) as xin,
                        tc.tile_pool(name="stage", bufs=2) as stage,
                        tc.tile_pool(name="rot", bufs=2) as rot_pool,
                        tc.tile_pool(name="tab", bufs=1) as tab,
                    ):
                        w_all = wq_pool.tile([128, KO, NM * 128], MDT)
                        nc.sync.dma_start(w_all, wqkvT[:].rearrange("k p m -> p k m"))
                        cos_sb = tab.tile([128, S], TDT)
                        nc.sync.dma_start(cos_sb, cosT[:])
                        sin_sb = tab.tile([128, S], TDT)
                        nc.sync.dma_start(sin_sb, sinT[:])

                        SLAB = TOK  # one gathered token-shard per slab
                        KH = min(8, KO)  # k-tiles per x-slab chunk
                        NCH = KO // KH
                        RH = min(256, SLAB)  # RoPE column-chunk
                        for n in range(cfg.T // SLAB):
                            tsl = ds(n * SLAB, SLAB)
                            # PSUM tiles first so matmuls can start per-chunk
                            pss = [
                                psum.tile([128, SLAB], F32, tag="bank", name=f"qk_ps{m}")
                                for m in range(NM)
                            ]
                            for ch in range(NCH):
                                xt = xin.tile([128, KH, SLAB], MDT, tag="xh")
                                src = (
                                    xTg1[n, ds(ch * KH, KH), :, :]
                                    if (ch * KH) < KO2
                                    else xTg2[n, ds(ch * KH - KO2, KH), :, :]
                                )
                                nc.sync.dma_start(xt, src.rearrange("k p t -> p k t"))
                                for m in range(NM):
                                    for k in range(KH):
                                        nc.tensor.matmul(
                                            pss[m],
                                            w_all[:, ch * KH + k, ts(m, 128)],
                                            xt[:, k, :],
                                            start=(ch == 0 and k == 0),
                                            stop=(ch == NCH - 1 and k == KH - 1),
                                        )
                            st = stage.tile([128, NM, SLAB], MDT)
                            for m in range(NM):
                                nc.scalar.copy(st[:, m, :], pss[m])
                            # RoPE on q heads + k head (rows 0..nq), not v
                            for rh in range(SLAB // RH):
                                rsl = ds(rh * RH, RH)
                                rot = rot_pool.tile([128, nq + 1, RH], MDT, tag="rot")
                                nc.sync.dma_start(rot[0:64], st[64:128, 0 : nq + 1, rsl])
                                nc.sync.dma_start(rot[64:128], st[0:64, 0 : nq + 1, rsl])
                                s0 = (n * SLAB + rh * RH) % S
                                cos_b = cos_sb[:, None, ds(s0, RH)].to_broadcast(
                                    (128, nq + 1, RH)
                                )
                                sin_b = sin_sb[:, None, ds(s0, RH)].to_broadcast(
                                    (128, nq + 1, RH)
                                )
                                nc.vector.tensor_mul(
                                    st[:, 0 : nq + 1, rsl], st[:, 0 : nq + 1, rsl], cos_b
                                )
                                nc.vector.tensor_mul(rot, rot, sin_b)
                                nc.vector.tensor_add(
                                    st[:, 0 : nq + 1, rsl], st[:, 0 : nq + 1, rsl], rot
                                )
                            nc.sync.dma_start(
                                qkv_sp[:, :, tsl].rearrange("m p t -> p m t"), st
                            )

                if 2 in phases:
                    # ------- phase 2: attention + per-batch o_proj + RS -------
                    with (
                        tc.tile_pool(name="attn", bufs=2) as attn_pool,
                        tc.tile_pool(name="kv", bufs=2) as kv_pool,
                        tc.tile_pool(name="vn", bufs=2) as vn_pool,
                        tc.tile_pool(name="qh", bufs=3) as q_pool,
                        tc.tile_pool(name="pt", bufs=12) as pt_pool,
                        tc.tile_pool(name="sm", bufs=4) as sm_pool,
                        tc.tile_pool(name="wo", bufs=1) as wo_pool,
                        tc.tile_pool(name="ot", bufs=4) as ot_pool,
                    ):
                        wo_all = wo_pool.tile([128, nq, H], MDT)
                        nc.sync.dma_start(wo_all, woT[:].rearrange("a p n -> p a n"))
                        for b in range(B):
                            bsl = ds(b * S, S)
                            attnT = attn_pool.tile([128, nq, S], MDT, tag="attnT")
                            kT = kv_pool.tile([128, S], MDT, tag="kT")
                            nc.sync.dma_start(kT, qkv_sp[nq, :, bsl])
                            vT = kv_pool.tile([128, S], MDT, tag="vT")
                            nc.sync.dma_start(vT, qkv_sp[nq + 1, :, bsl])
                            vn = vn_pool.tile([128, NKT, 128], MDT)
                            for kt in range(NKT):
                                tp = psum.tile([128, 128], MDT, tag="bank")
                                nc.tensor.transpose(tp, vT[:, ts(kt, 128)], ident)
                                nc.vector.tensor_copy(vn[:, kt, :], tp)
                            for h in range(nq):
                                q = q_pool.tile([128, S], MDT)
                                nc.sync.dma_start(q, qkv_sp[h, :, bsl])
                                for j in range(NJ):
                                    nkt = (j + 1) * NRT
                                    qsl = ds(j * QW, QW)
                                    pts = []
                                    for kt in range(nkt):
                                        sp = psum.tile([128, QW], F32, tag="bank")
                                        nc.tensor.matmul(
                                            sp,
                                            kT[:, ts(kt, 128)],
                                            q[:, qsl],
                                            start=True,
                                            stop=True,
                                        )
                                        pt = pt_pool.tile([128, QW], MDT, tag="pt")
                                        nc.scalar.activation(pt, sp, EXPF, bias=nbias[:, 0:1], scale=scale)
                                        if kt >= j * NRT:
                                            nc.vector.tensor_mul(
                                                pt, pt, masks_sb[:, kt - j * NRT, :]
                                            )
                                        pts.append(pt)
                                    sum_ps = psum.tile([1, QW], F32, tag="bank")
                                    for i, pt in enumerate(pts):
                                        nc.tensor.matmul(
                                            sum_ps,
                                            ones,
                                            pt,
                                            start=(i == 0),
                                            stop=(i == len(pts) - 1),
                                        )
                                    recip = sm_pool.tile([1, QW], F32, tag="recip")
                                    nc.vector.reciprocal(recip, sum_ps)
                                    pv = psum.tile([128, QW], F32, tag="bank")
                                    for i, pt in enumerate(pts):
                                        nc.tensor.matmul(
                                            pv,
                                            vn[:, i, :],
                                            pt,
                                            start=(i == 0),
                                            stop=(i == len(pts) - 1),
                                        )
                                    # broadcast 1/denom across partitions via a
                                    # rank-1 PE matmul, emitted after the PV
                                    # accumulation so the reciprocal is ready
                                    # (keeps gpsimd free for the CC triggers)
                                    rb_ps = psum.tile([128, QW], F32, tag="bank")
                                    nc.tensor.matmul(
                                        rb_ps, ones_row, recip, start=True, stop=True
                                    )
                                    rb = sm_pool.tile([128, QW], F32, tag="rb")
                                    nc.scalar.copy(rb, rb_ps)
                                    nc.vector.tensor_mul(
                                        attnT[:, h, ds(j * QW, QW)], pv, rb
                                    )

                            if 3 in phases:
                                # ---- o_proj rows for this batch ----
                                for tm in range(S // 128):
                                    tloc = tm * 128
                                    for nh in range(H // 512):
                                        hsl = ds(nh * 512, 512)
                                        ps = psum.tile([128, 512], F32, tag="bank")
                                        for a in range(nq):
                                            nc.tensor.matmul(
                                                ps,
                                                attnT[:, a, ds(tloc, 128)],
                                                wo_all[:, a, hsl],
                                                start=(a == 0),
                                                stop=(a == nq - 1),
                                            )
                                        ot = ot_pool.tile([128, 512], F16)
                                        if (tm + nh) % 2 == 0:
                                            nc.scalar.copy(ot, ps)
                                        else:
                                            nc.vector.tensor_copy(ot, ps)
                                        nc.sync.dma_start(
                                            o_sp[ds(b * S + tloc, 128), hsl], ot
                                        )
                                # ---- ReduceScatter this batch's partials ----
                                nc.gpsimd.collective_compute(
                                    "ReduceScatter",
                                    mybir.AluOpType.add,
                                    replica_groups=groups,
                                    ins=[o_sp[ds(b * S, S), :].opt()],
                                    outs=[b_rso[ds(b * 128, 128), :].opt()],
                                )
                                nc.sync.dma_start(
                                    out[ds(b * 128, 128), :],
                                    b_rso[ds(b * 128, 128), :],
                                )
    nc.compile()
    return nc


def prep_core_inputs(cfg: Cfg, hidden, w_qkv, w_o, core: int):
    """Build the per-core input map (C-contiguous, matmul-path dtype)."""
    ndt = np.float16 if cfg.mm16 else np.float32
    T, H, S, nq, D = cfg.T, cfg.H, cfg.S, cfg.nq, cfg.D
    TOK = cfg.TOK
    NQ_TOT = cfg.n_cores * nq
    xT = np.ascontiguousarray(
        hidden[core * TOK : (core + 1) * TOK, :].T.astype(ndt)
    ).reshape(cfg.KO, 128, TOK)
    q0 = core * nq * D
    rows = list(range(q0, q0 + nq * D))
    rows += list(range(NQ_TOT * D + core * D, NQ_TOT * D + (core + 1) * D))
    kv_heads = cfg.n_cores  # one kv head per core
    rows += list(
        range((NQ_TOT + kv_heads) * D + core * D, (NQ_TOT + kv_heads) * D + (core + 1) * D)
    )
    wqkvT = np.ascontiguousarray(w_qkv[rows, :].T.astype(ndt)).reshape(cfg.KO, 128, cfg.NM * 128)
    woT = np.ascontiguousarray(
        w_o[:, core * nq * D : (core + 1) * nq * D].T.astype(ndt)
    ).reshape(nq, 128, H)
    return {"xT": xT, "wqkvT": wqkvT, "woT": woT}


def prep_shared_inputs(cfg: Cfg, rope_theta=10000.0):
    ndt = np.float16 if cfg.mm16 else np.float32
    S, D = cfg.S, cfg.D
    inv = 1.0 / (rope_theta ** (np.arange(0, D, 2, dtype=np.float64) / D))
    ang = np.arange(S, dtype=np.float64)[:, None] * inv[None, :]  # [S, 64]
    cos = np.cos(ang).T.astype(np.float32)  # [64, S]
    sin = np.sin(ang).T.astype(np.float32)
    cosT = np.concatenate([cos, cos], axis=0)  # [128, S]
    sinT = np.concatenate([-sin, sin], axis=0)
    return {
        "cosT": np.ascontiguousarray(cosT.astype(ndt)),
        "sinT": np.ascontiguousarray(sinT.astype(ndt)),
    }


_CACHE = {}
LAST_EXEC_NS = None


def _get_nc(cfg: Cfg) -> bass.Bass:
    if cfg not in _CACHE:
        _CACHE[cfg] = build_nc(cfg)
    return _CACHE[cfg]


def kernel(hidden_states=None, w_qkv=None, w_o=None, seq_len=None, **_):
    cfg = FULL
    hidden = np.asarray(hidden_states, dtype=np.float32)
    w_qkv = np.asarray(w_qkv, dtype=np.float32)
    w_o = np.asarray(w_o, dtype=np.float32)

    nc = _get_nc(cfg)
    shared = prep_shared_inputs(cfg)
    in_maps = []
    for c in range(cfg.n_cores):
        m = dict(shared)
        m.update(prep_core_inputs(cfg, hidden, w_qkv, w_o, c))
        in_maps.append(m)

    trace = os.environ.get("KERNEL_TRACE", "0") == "1"
    res = run_bass_kernel_spmd(
        nc, in_maps, core_ids=list(range(cfg.n_cores)), trace=trace
    )
    global LAST_EXEC_NS
    if res.exec_time_ns is not None:
        LAST_EXEC_NS = res.exec_time_ns
        print(f"HW exec time: {res.exec_time_ns} ns")
        if res.instructions_and_trace is not None:
            print(f"trace: {res.instructions_and_trace[1]}")
    # Reassemble: RS chunk b on core c holds final rows [S*b + 128*c, +128).
    out_full = np.empty((cfg.T, cfg.H), dtype=np.float32)
    for c in range(cfg.n_cores):
        sh = res.results[c]["o_sh"]
        for b in range(cfg.B):
            out_full[cfg.S * b + 128 * c : cfg.S * b + 128 * c + 128] = sh[
                128 * b : 128 * b + 128
            ]
    return out_full


# revision 11
# speedup vs baseline: 1.1524x; 1.0379x over previous
# Mistral-style GQA attention layer (QKV proj + RoPE + causal attention +
# o_proj), tensor-parallel over heads across 8 NeuronCores.
#
# Sharding (8-way TP over heads): core c owns q heads [4c..4c+4) and kv head c.
#   - w_qkv rows sharded: 4 q-head blocks + 1 k block + 1 v block per core
#   - w_o columns sharded: each core computes a partial o_proj output.
#
# Host<->device traffic is minimized: every input byte is uploaded exactly
# once (fp16), and every output byte downloaded exactly once:
#   - x is uploaded token-sharded (1/8 per core) and AllGathered on device
#     (split in two along the contraction axis so phase 1 can start on the
#     first half while the second gathers).
#   - o_proj partials are ReduceScattered on device per batch (4 chunks,
#     pipelined behind compute), so each core returns a 512-row scramble of
#     the final output that the host reassembles by slicing.
#   - identity/ones/causal-mask tables are generated on device.
#
# Device kernel (identical SPMD program, per-core data):
#   phase 0: 2x AllGather of the token-shards of x^T.
#   phase 1: qkvT = Wc @ X^T (outputs TRANSPOSED: [dim, t]) + inline RoPE on
#            q/k rows, spilled to DRAM scratch.
#   phase 2 (per batch): per head: S^T = K Q^T on PE, exp on ACT (no max-sub:
#            scores are O(5) and fp32 exp is safe), causal mask via
#            multiplicative 0/1 tiles on DVE, softmax denominator via
#            ones-matmul over the partition (key) axis, P@V with V as the
#            stationary operand (needs V natural layout -> 128x128 PE
#            transposes of V^T), normalize at the end. Then o_partial rows
#            for this batch (attn @ Wo_c^T) and the batch's ReduceScatter.
#
# All matmuls run in fp16 (full PE rate, fp32 PSUM accumulate); inputs are
# quantized to fp16 on host (~1e-3 relative error, well inside tolerance).

import os
from contextlib import nullcontext
from dataclasses import dataclass

import numpy as np

import concourse.bass as bass
from concourse import bacc
import concourse.mybir as mybir
import concourse.tile as tile
from concourse.bass import ds, ts
from concourse.bass_utils import run_bass_kernel_spmd
from concourse.masks import make_identity

F32 = mybir.dt.float32
F32R = mybir.dt.float32r
EXPF = mybir.ActivationFunctionType.Exp
F16 = mybir.dt.float16


@dataclass(frozen=True)
class Cfg:
    T: int = 4096          # total tokens (B*S)
    H: int = 4096          # hidden size
    S: int = 1024          # seq len
    nq: int = 4            # q heads per core
    n_cores: int = 8
    D: int = 128           # head dim
    mm16: bool = True      # fp16 matmul paths instead of fp32r

    @property
    def B(self):
        return self.T // self.S

    @property
    def TOK(self):  # token shard per core
        return self.T // self.n_cores

    @property
    def KO(self):  # contraction tiles for qkv proj
        return self.H // 128

    @property
    def NM(self):  # qkv output row-tiles per core (q heads + k + v)
        return self.nq + 2

    @property
    def QW(self):  # q tile width in attention
        return min(512, self.S)

    @property
    def NJ(self):
        return self.S // self.QW

    @property
    def NKT(self):  # key tiles per batch
        return self.S // 128


FULL = Cfg()


def build_nc(cfg: Cfg, loop: int | None = None, phases=(0, 1, 2, 3)) -> bass.Bass:
    nc = bacc.Bacc("TRN2", target_bir_lowering=False, debug=False, num_devices=cfg.n_cores)
    MDT = F16 if cfg.mm16 else F32R          # matmul-path storage dtype
    TDT = F16 if cfg.mm16 else F32           # table dtype (cos/sin/masks)
    T, H, S, nq, D = cfg.T, cfg.H, cfg.S, cfg.nq, cfg.D
    KO, NM, QW, NJ, NKT, B = cfg.KO, cfg.NM, cfg.QW, cfg.NJ, cfg.NKT, cfg.B
    TOK, NC = cfg.TOK, cfg.n_cores
    KO2 = KO // 2  # AllGather half (k-tiles)
    NRT = QW // 128  # number of diagonal mask offsets
    scale = 1.0 / np.sqrt(D)
    groups = [list(range(NC))]

    xT = nc.dram_tensor("xT", [KO, 128, TOK], MDT, kind="ExternalInput")
    wqkvT = nc.dram_tensor("wqkvT", [KO, 128, NM * 128], MDT, kind="ExternalInput")
    woT = nc.dram_tensor("woT", [nq, 128, H], MDT, kind="ExternalInput")
    cosT = nc.dram_tensor("cosT", [128, S], TDT, kind="ExternalInput")
    sinT = nc.dram_tensor("sinT", [128, S], TDT, kind="ExternalInput")
    out = nc.dram_tensor("o_sh", [TOK, H], F16, kind="ExternalOutput")

    with tile.TileContext(nc) as tc:
        with (
            tc.tile_pool(name="psum", bufs=8, space="PSUM") as psum,
            tc.tile_pool(name="consts", bufs=1) as consts,
            tc.tile_pool(name="dram", bufs=1, space="DRAM") as dram,
        ):
            qkv_sp = dram.tile([NM, 128, T], MDT)
            xTg1 = dram.tile([NC, KO2, 128, TOK], MDT, addr_space="Shared")
            xTg2 = dram.tile([NC, KO2, 128, TOK], MDT, addr_space="Shared")
            o_sp = dram.tile([T, H], F16)
            b_rso = dram.tile([TOK, H], F16)

            with (tc.For_i(0, loop, 1) if loop else nullcontext()):
                if 0 in phases:
                    # -------- phase 0: AllGather token-shards of x^T --------
                    b_xin = dram.tile([KO, 128, TOK], MDT, tag="b_xin")
                    nc.sync.dma_start(b_xin[ds(0, KO2)], xT[ds(0, KO2)])
                    nc.sync.dma_start(b_xin[ds(KO2, KO2)], xT[ds(KO2, KO2)])
                    nc.gpsimd.collective_compute(
                        "AllGather",
                        mybir.AluOpType.bypass,
                        replica_groups=groups,
                        ins=[b_xin[ds(0, KO2), :, :].opt()],
                        outs=[xTg1[:].opt()],
                    )
                    nc.gpsimd.collective_compute(
                        "AllGather",
                        mybir.AluOpType.bypass,
                        replica_groups=groups,
                        ins=[b_xin[ds(KO2, KO2), :, :].opt()],
                        outs=[xTg2[:].opt()],
                    )

                # device-generated tables (after the CC triggers so the
                # gathers start as early as possible)
                ident = consts.tile([128, 128], MDT)
                make_identity(nc, ident)
                ones = consts.tile([128, 1], MDT)
                nc.gpsimd.memset(ones, 1.0)
                masks_sb = consts.tile([128, NRT, QW], TDT)
                nc.gpsimd.memset(masks_sb, 1.0)
                for r in range(NRT):
                    # masks_sb[p, r, q] = (q - p - 128 r) >= 0 ? 1 : 0
                    nc.gpsimd.affine_select(
                        out=masks_sb[:, r, :],
                        in_=masks_sb[:, r, :],
                        compare_op=mybir.AluOpType.is_ge,
                        fill=0.0,
                        base=-128 * r,
                        pattern=[[1, QW]],
                        channel_multiplier=-1,
                    )
                nbias = consts.tile([128, 1], F32)
                nc.gpsimd.memset(nbias, -4.0)

                if 1 in phases:
                    # ---------------- phase 1: QKV projection + RoPE ----------------
                    with (
                        tc.tile_pool(name="wq", bufs=1) as wq_pool,
                        tc.tile_pool(name="xin", bufs=2) as xin,
                        tc.tile_pool(name="stage", bufs=2) as stage,
                        tc.tile_pool(name="rot", bufs=2) as rot_pool,
                        tc.tile_pool(name="tab", bufs=1) as tab,
                    ):
                        w_all = wq_pool.tile([128, KO, NM * 128], MDT)
                        nc.sync.dma_start(w_all, wqkvT[:].rearrange("k p m -> p k m"))
                        cos_sb = tab.tile([128, S], TDT)
                        nc.sync.dma_start(cos_sb, cosT[:])
                        sin_sb = tab.tile([128, S], TDT)
                        nc.sync.dma_start(sin_sb, sinT[:])

                        SLAB = TOK  # one gathered token-shard per slab
                        KH = min(8, KO)  # k-tiles per x-slab chunk
                        NCH = KO // KH
                        RH = min(256, SLAB)  # RoPE column-chunk
                        for n in range(cfg.T // SLAB):
                            tsl = ds(n * SLAB, SLAB)
                            # PSUM tiles first so matmuls can start per-chunk
                            pss = [
                                psum.tile([128, SLAB], F32, tag="bank", name=f"qk_ps{m}")
                                for m in range(NM)
                            ]
                            for ch in range(NCH):
                                xt = xin.tile([128, KH, SLAB], MDT, tag="xh")
                                src = (
                                    xTg1[n, ds(ch * KH, KH), :, :]
                                    if (ch * KH) < KO2
                                    else xTg2[n, ds(ch * KH - KO2, KH), :, :]
                                )
                                nc.sync.dma_start(xt, src.rearrange("k p t -> p k t"))
                                for m in range(NM):
                                    for k in range(KH):
                                        nc.tensor.matmul(
                                            pss[m],
                                            w_all[:, ch * KH + k, ts(m, 128)],
                                            xt[:, k, :],
                                            start=(ch == 0 and k == 0),
                                            stop=(ch == NCH - 1 and k == KH - 1),
                                        )
                            st = stage.tile([128, NM, SLAB], MDT)
                            for m in range(NM):
                                nc.scalar.copy(st[:, m, :], pss[m])
                            # RoPE on q heads + k head (rows 0..nq), not v
                            for rh in range(SLAB // RH):
                                rsl = ds(rh * RH, RH)
                                rot = rot_pool.tile([128, nq + 1, RH], MDT, tag="rot")
                                nc.sync.dma_start(rot[0:64], st[64:128, 0 : nq + 1, rsl])
                                nc.sync.dma_start(rot[64:128], st[0:64, 0 : nq + 1, rsl])
                                s0 = (n * SLAB + rh * RH) % S
                                cos_b = cos_sb[:, None, ds(s0, RH)].to_broadcast(
                                    (128, nq + 1, RH)
                                )
                                sin_b = sin_sb[:, None, ds(s0, RH)].to_broadcast(
                                    (128, nq + 1, RH)
                                )
                                nc.vector.tensor_mul(
                                    st[:, 0 : nq + 1, rsl], st[:, 0 : nq + 1, rsl], cos_b
                                )
                                nc.vector.tensor_mul(rot, rot, sin_b)
                                nc.vector.tensor_add(
                                    st[:, 0 : nq + 1, rsl], st[:, 0 : nq + 1, rsl], rot
                                )
                            nc.sync.dma_start(
                                qkv_sp[:, :, tsl].rearrange("m p t -> p m t"), st
                            )

                if 2 in phases:
                    # ------- phase 2: attention + per-batch o_proj + RS -------
                    with (
                        tc.tile_pool(name="attn", bufs=2) as attn_pool,
                        tc.tile_pool(name="kv", bufs=2) as kv_pool,
                        tc.tile_pool(name="vn", bufs=2) as vn_pool,
                        tc.tile_pool(name="qh", bufs=3) as q_pool,
                        tc.tile_pool(name="pt", bufs=12) as pt_pool,
                        tc.tile_pool(name="sm", bufs=4) as sm_pool,
                        tc.tile_pool(name="wo", bufs=1) as wo_pool,
                        tc.tile_pool(name="ot", bufs=4) as ot_pool,
                    ):
                        wo_all = wo_pool.tile([128, nq, H], MDT)
                        nc.sync.dma_start(wo_all, woT[:].rearrange("a p n -> p a n"))
                        for b in range(B):
                            bsl = ds(b * S, S)
                            attnT = attn_pool.tile([128, nq, S], MDT, tag="attnT")
                            kT = kv_pool.tile([128, S], MDT, tag="kT")
                            nc.sync.dma_start(kT, qkv_sp[nq, :, bsl])
                            vT = kv_pool.tile([128, S], MDT, tag="vT")
                            nc.sync.dma_start(vT, qkv_sp[nq + 1, :, bsl])
                            vn = vn_pool.tile([128, NKT, 128], MDT)
                            for kt in range(NKT):
                                tp = psum.tile([128, 128], MDT, tag="bank")
                                nc.tensor.transpose(tp, vT[:, ts(kt, 128)], ident)
                                nc.vector.tensor_copy(vn[:, kt, :], tp)
                            for h in range(nq):
                                q = q_pool.tile([128, S], MDT)
                                nc.sync.dma_start(q, qkv_sp[h, :, bsl])
                                for j in range(NJ):
                                    nkt = (j + 1) * NRT
                                    qsl = ds(j * QW, QW)
                                    pts = []
                                    for kt in range(nkt):
                                        sp = psum.tile([128, QW], F32, tag="bank")
                                        nc.tensor.matmul(
                                            sp,
                                            kT[:, ts(kt, 128)],
                                            q[:, qsl],
                                            start=True,
                                            stop=True,
                                        )
                                        pt = pt_pool.tile([128, QW], MDT, tag="pt")
                                        nc.scalar.activation(pt, sp, EXPF, bias=nbias[:, 0:1], scale=scale)
                                        if kt >= j * NRT:
                                            nc.vector.tensor_mul(
                                                pt, pt, masks_sb[:, kt - j * NRT, :]
                                            )
                                        pts.append(pt)
                                    sum_ps = psum.tile([1, QW], F32, tag="bank")
                                    for i, pt in enumerate(pts):
                                        nc.tensor.matmul(
                                            sum_ps,
                                            ones,
                                            pt,
                                            start=(i == 0),
                                            stop=(i == len(pts) - 1),
                                        )
                                    recip = sm_pool.tile([1, QW], F32, tag="recip")
                                    nc.vector.reciprocal(recip, sum_ps)
                                    pv = psum.tile([128, QW], F32, tag="bank")
                                    for i, pt in enumerate(pts):
                                        nc.tensor.matmul(
                                            pv,
                                            vn[:, i, :],
                                            pt,
                                            start=(i == 0),
                                            stop=(i == len(pts) - 1),
                                        )
                                    rb = sm_pool.tile([128, QW], F32, tag="rb")
                                    nc.gpsimd.partition_broadcast(rb, recip)
                                    nc.vector.tensor_mul(
                                        attnT[:, h, ds(j * QW, QW)], pv, rb
                                    )

                            if 3 in phases:
                                # ---- o_proj rows for this batch ----
                                for tm in range(S // 128):
                                    tloc = tm * 128
                                    for nh in range(H // 512):
                                        hsl = ds(nh * 512, 512)
                                        ps = psum.tile([128, 512], F32, tag="bank")
                                        for a in range(nq):
                                            nc.tensor.matmul(
                                                ps,
                                                attnT[:, a, ds(tloc, 128)],
                                                wo_all[:, a, hsl],
                                                start=(a == 0),
                                                stop=(a == nq - 1),
                                            )
                                        ot = ot_pool.tile([128, 512], F16)
                                        if (tm + nh) % 2 == 0:
                                            nc.scalar.copy(ot, ps)
                                        else:
                                            nc.vector.tensor_copy(ot, ps)
                                        nc.sync.dma_start(
                                            o_sp[ds(b * S + tloc, 128), hsl], ot
                                        )
                                # ---- ReduceScatter this batch's partials ----
                                nc.gpsimd.collective_compute(
                                    "ReduceScatter",
                                    mybir.AluOpType.add,
                                    replica_groups=groups,
                                    ins=[o_sp[ds(b * S, S), :].opt()],
                                    outs=[b_rso[ds(b * 128, 128), :].opt()],
                                )
                                nc.sync.dma_start(
                                    out[ds(b * 128, 128), :],
                                    b_rso[ds(b * 128, 128), :],
                                )
    nc.compile()
    return nc


def prep_core_inputs(cfg: Cfg, hidden, w_qkv, w_o, core: int):
    """Build the per-core input map (C-contiguous, matmul-path dtype)."""
    ndt = np.float16 if cfg.mm16 else np.float32
    T, H, S, nq, D = cfg.T, cfg.H, cfg.S, cfg.nq, cfg.D
    TOK = cfg.TOK
    NQ_TOT = cfg.n_cores * nq
    xT = np.ascontiguousarray(
        hidden[core * TOK : (core + 1) * TOK, :].T.astype(ndt)
    ).reshape(cfg.KO, 128, TOK)
    q0 = core * nq * D
    rows = list(range(q0, q0 + nq * D))
    rows += list(range(NQ_TOT * D + core * D, NQ_TOT * D + (core + 1) * D))
    kv_heads = cfg.n_cores  # one kv head per core
    rows += list(
        range((NQ_TOT + kv_heads) * D + core * D, (NQ_TOT + kv_heads) * D + (core + 1) * D)
    )
    wqkvT = np.ascontiguousarray(w_qkv[rows, :].T.astype(ndt)).reshape(cfg.KO, 128, cfg.NM * 128)
    woT = np.ascontiguousarray(
        w_o[:, core * nq * D : (core + 1) * nq * D].T.astype(ndt)
    ).reshape(nq, 128, H)
    return {"xT": xT, "wqkvT": wqkvT, "woT": woT}


def prep_shared_inputs(cfg: Cfg, rope_theta=10000.0):
    ndt = np.float16 if cfg.mm16 else np.float32
    S, D = cfg.S, cfg.D
    inv = 1.0 / (rope_theta ** (np.arange(0, D, 2, dtype=np.float64) / D))
    ang = np.arange(S, dtype=np.float64)[:, None] * inv[None, :]  # [S, 64]
    cos = np.cos(ang).T.astype(np.float32)  # [64, S]
    sin = np.sin(ang).T.astype(np.float32)
    cosT = np.concatenate([cos, cos], axis=0)  # [128, S]
    sinT = np.concatenate([-sin, sin], axis=0)
    return {
        "cosT": np.ascontiguousarray(cosT.astype(ndt)),
        "sinT": np.ascontiguousarray(sinT.astype(ndt)),
    }


_CACHE = {}
LAST_EXEC_NS = None


def _get_nc(cfg: Cfg) -> bass.Bass:
    if cfg not in _CACHE:
        _CACHE[cfg] = build_nc(cfg)
    return _CACHE[cfg]


def kernel(hidden_states=None, w_qkv=None, w_o=None, seq_len=None, **_):
    cfg = FULL
    hidden = np.asarray(hidden_states, dtype=np.float32)
    w_qkv = np.asarray(w_qkv, dtype=np.float32)
    w_o = np.asarray(w_o, dtype=np.float32)

    nc = _get_nc(cfg)
    shared = prep_shared_inputs(cfg)
    in_maps = []
    for c in range(cfg.n_cores):
        m = dict(shared)
        m.update(prep_core_inputs(cfg, hidden, w_qkv, w_o, c))
        in_maps.append(m)

    trace = os.environ.get("KERNEL_TRACE", "0") == "1"
    res = run_bass_kernel_spmd(
        nc, in_maps, core_ids=list(range(cfg.n_cores)), trace=trace
    )
    global LAST_EXEC_NS
    if res.exec_time_ns is not None:
        LAST_EXEC_NS = res.exec_time_ns
        print(f"HW exec time: {res.exec_time_ns} ns")
        if res.instructions_and_trace is not None:
            print(f"trace: {res.instructions_and_trace[1]}")
    # Reassemble: RS chunk b on core c holds final rows [S*b + 128*c, +128).
    out_full = np.empty((cfg.T, cfg.H), dtype=np.float32)
    for c in range(cfg.n_cores):
        sh = res.results[c]["o_sh"]
        for b in range(cfg.B):
            out_full[cfg.S * b + 128 * c : cfg.S * b + 128 * c + 128] = sh[
                128 * b : 128 * b + 128
            ]
    return out_full
